# revision 19
# baseline (speedup 1.0000x reference)
"""Trainium2 Bass kernel for nn_DecNP_6012954214675 (2-stage PointNet++ feature
propagation / kNN-interpolation decoder).

Sharding: 8 cores; core c handles batch b = c//2 and half h = c%2 of the fine
point axis (N=8192 -> 4096 per core). Stage 1 (2048 queries over 512 supports)
is replicated on both cores of a batch; its result is the stage-2 gather table.

Numerics: the reference computes d = fl(fl(|q|^2+|p|^2) - 2*fl(q.p)) with
fl(q.p) produced by the XLA fp32 K=3 matmul, whose bit pattern equals a bass
fp32 PE matmul (verified on HW). We reproduce -d bitwise per tile as
  negd = fl(fl(-B + -A) + 2*dot_PE)        (scalar_tensor_tensor, one DVE op)
with host-prepped -A (per query), -B broadcast (per support), and queries
pre-doubled so PSUM holds exactly 2*dot (x2 is exact). max8/max_index on negd
then reproduce the reference's top-3 selection and its distance values, so
weights match the reference to ~1 ulp even at near-coincident points.

Schedule: tiles are processed in groups with a two-pass software pipeline —
pass A (score matmuls -> negd -> top-3 -> weights -> indirect gathers + diag
build) runs one group ahead of pass B (diag-matmul weighted sum -> PSUM ->
PE transpose -> output DMA), so the PE never stalls on a tile's gather
latency and stays HAM-warm. Score PSUM is chunked [128, 1024] so chunk c+1's
matmuls overlap chunk c's scalar_tensor_tensor drain.
"""

import numpy as np

import concourse.bass as bass
import concourse.mybir as mybir
import concourse.tile as tile
from concourse import bacc
from concourse.bass_utils import run_bass_kernel_spmd
from concourse.masks import make_identity

F32 = mybir.dt.float32
U32 = mybir.dt.uint32
ALU = mybir.AluOpType

B, N0, N1, S1 = 4, 8192, 2048, 512
D0, D1, D2 = 128, 256, 512          # x0 / x1 / x2 feature dims
DT2 = D1 + D2                        # 768: stage-2 table feature dim
NLOC = N0 // 2                       # 4096 fine queries per core
T1 = N1 // 128                       # 16 stage-1 tiles
T2 = NLOC // 128                     # 32 stage-2 tiles
G = 4                                # pipeline group size (tiles)
N_CORES = 8

_PROGRAM = None


def _emit(tc, ctx):
    nc = tc.nc
    ts = bass.ts

    q1e = nc.dram_tensor("q1e", [3, N1], F32, kind="ExternalInput")
    r1 = nc.dram_tensor("r1", [3, S1], F32, kind="ExternalInput")
    qn1 = nc.dram_tensor("qn1", [128, T1], F32, kind="ExternalInput")
    b1r = nc.dram_tensor("b1r", [128, S1], F32, kind="ExternalInput")
    q2e = nc.dram_tensor("q2e", [128, NLOC], F32, kind="ExternalInput")
    r2 = nc.dram_tensor("r2", [128, 512], F32, kind="ExternalInput")
    qn2 = nc.dram_tensor("qn2", [128, T2], F32, kind="ExternalInput")
    b2r = nc.dram_tensor("b2r", [128, N1], F32, kind="ExternalInput")
    x2t = nc.dram_tensor("x2t", [S1, D2], F32, kind="ExternalInput")
    x1t = nc.dram_tensor("x1t", [N1, D1], F32, kind="ExternalInput")
    t2 = nc.dram_tensor("t2", [N1, DT2], F32)
    o2t = nc.dram_tensor("o2t", [DT2, NLOC], F32, kind="ExternalOutput")

    cst = ctx.enter_context(tc.tile_pool(name="cst", bufs=1))
    sm = ctx.enter_context(tc.tile_pool(name="sm", bufs=10))
    ndp = ctx.enter_context(tc.tile_pool(name="ndp", bufs=3))
    gat = ctx.enter_context(tc.tile_pool(name="gat", bufs=6 * G))
    dia = ctx.enter_context(tc.tile_pool(name="dia", bufs=6 * G))
    isb = ctx.enter_context(tc.tile_pool(name="isb", bufs=2))
    trs = ctx.enter_context(tc.tile_pool(name="trs", bufs=2))
    bnc = ctx.enter_context(tc.tile_pool(name="bnc", bufs=2))
    ps_sc = ctx.enter_context(tc.tile_pool(name="ps_sc", bufs=4, space="PSUM"))
    ps_ip = ctx.enter_context(tc.tile_pool(name="ps_ip", bufs=1, space="PSUM"))
    ps_tr = ctx.enter_context(tc.tile_pool(name="ps_tr", bufs=1, space="PSUM"))

    ident = cst.tile([128, 128], F32)
    make_identity(nc, ident[:])

    def load(handle, shape, tag):
        t = cst.tile(shape, F32, tag=tag)
        nc.sync.dma_start(t[:], handle.ap())
        return t

    q1s = load(q1e, [3, N1], "q1s")
    r1s = load(r1, [3, S1], "r1s")
    qn1s = load(qn1, [128, T1], "qn1s")
    q2s = load(q2e, [128, NLOC], "q2s")   # rows 32g+c = 2*coord c (replicated)
    r2s = load(r2, [128, 512], "r2s")     # rows 32g+c = coord c of chunk g
    qn2s = load(qn2, [128, T2], "qn2s")
    nb1 = load(b1r, [128, S1], "nb1")    # -B, host pre-broadcast
    nb2 = load(b2r, [128, N1], "nb2")

    # x1^T -> t2[:, 0:256] (SBUF bounce)
    for t in range(T1):
        xb = bnc.tile([128, D1], F32, tag="bounce")
        nc.sync.dma_start(xb[:], x1t.ap()[ts(t, 128), :])
        nc.sync.dma_start(t2.ap()[ts(t, 128), 0:D1], xb[:])

    def pass_a(t, qs, rs, nbs, qns, swide, table, dfeat):
        """scores -> exact negd -> top-3 -> weights -> gathers + diag build."""
        negd = ndp.tile([128, swide], F32, tag="negd")
        for c0 in range(0, swide, 1024):
            cw = min(1024, swide - c0)
            scch = ps_sc.tile([128, cw], F32, tag="sc")
            for j0 in range(0, cw, 512):
                nc.tensor.matmul(out=scch[:, j0:j0 + 512],
                                 lhsT=qs[:, ts(t, 128)],
                                 rhs=rs[:, c0 + j0:c0 + j0 + 512],
                                 start=True, stop=True)
            nc.vector.scalar_tensor_tensor(
                out=negd[:, c0:c0 + cw], in0=nbs[:, c0:c0 + cw],
                scalar=qns[:, t:t + 1], in1=scch[:],
                op0=ALU.add, op1=ALU.add,
            )
        return topk_tail(negd, table, dfeat)

    def topk_tail(negd, table, dfeat):
        mx = sm.tile([128, 8], F32, tag="mx")
        nc.vector.max(out=mx[:], in_=negd[:])
        ix = sm.tile([128, 8], U32, tag="ix")
        nc.vector.max_index(ix[:], mx[:], negd[:])
        dw = sm.tile([128, 3], F32, tag="dw")
        nc.gpsimd.tensor_scalar(
            out=dw[:], in0=mx[:, 0:3], scalar1=-1.0, scalar2=1e-8,
            op0=ALU.mult, op1=ALU.add,
        )
        rc = sm.tile([128, 3], F32, tag="rc")
        nc.vector.reciprocal(rc[:], dw[:])
        rs_ = sm.tile([128, 1], F32, tag="rs")
        nc.vector.tensor_reduce(out=rs_[:], in_=rc[:], axis=mybir.AxisListType.X,
                                op=ALU.add)
        rsr = sm.tile([128, 1], F32, tag="rsr")
        nc.vector.reciprocal(rsr[:], rs_[:])
        w = sm.tile([128, 3], F32, tag="w")
        nc.gpsimd.tensor_scalar(out=w[:], in0=rc[:], scalar1=rsr[:, 0:1],
                                scalar2=None, op0=ALU.mult)
        gs, dgs = [], []
        for k in range(3):
            g = gat.tile([128, dfeat], F32, tag="gather")
            nc.gpsimd.indirect_dma_start(
                out=g[:], out_offset=None, in_=table.ap(),
                in_offset=bass.IndirectOffsetOnAxis(ap=ix[:, k:k + 1], axis=0),
            )
            gs.append(g)
            if k > 0:     # k=0 is applied on the scalar engine in pass B
                dg = dia.tile([128, 128], F32, tag="diag")
                nc.gpsimd.affine_select(
                    out=dg[:], in_=w[:, k:k + 1].to_broadcast([128, 128]),
                    compare_op=ALU.is_equal, fill=0.0,
                    base=0, pattern=[[-1, 128]], channel_multiplier=1,
                )
                dgs.append(dg)
        return gs, dgs, w

    def wsum(st, dfeat):
        """row = fl(fl(w1*g1 + w2*g2) + w0*g0): k=1,2 on PE (diag matmuls into
        PSUM), k=0 on the scalar engine (scale-copy), combined by one DVE add."""
        gs, dgs, w = st
        s0 = trs.tile([128, dfeat], F32, tag="s0")
        nc.scalar.activation(out=s0[:], in_=gs[0][:],
                             func=mybir.ActivationFunctionType.Copy,
                             scale=w[:, 0:1])
        ip = ps_ip.tile([128, dfeat], F32, tag="ip")
        for k in (1, 2):
            for j0 in range(0, dfeat, 512):
                j1 = min(j0 + 512, dfeat)
                nc.tensor.matmul(out=ip[:, j0:j1], lhsT=dgs[k - 1][:],
                                 rhs=gs[k][:, j0:j1],
                                 start=(k == 1), stop=(k == 2))
        row = isb.tile([128, dfeat], F32, tag="isb")
        nc.vector.tensor_tensor(out=row[:], in0=ip[:], in1=s0[:], op=ALU.add)
        return row

    def pass_b1(t, st):
        row = wsum(st, D2)
        nc.sync.dma_start(t2.ap()[ts(t, 128), D1:DT2], row[:])

    def pass_b2(t, st):
        row = wsum(st, DT2)
        tr = ps_tr.tile([128, DT2], F32, tag="tr")
        for j in range(DT2 // 128):
            nc.tensor.transpose(tr[:, ts(j, 128)], row[:, ts(j, 128)], ident[:])
        trs_t = trs.tile([128, DT2], F32, tag="trs")
        nc.scalar.copy(trs_t[:], tr[:])
        dst = o2t.ap().rearrange("(j r) p -> r j p", r=128)[:, :, ts(t, 128)]
        src = trs_t[:].rearrange("r (j p) -> r j p", p=128)
        nc.sync.dma_start(dst, src)

    def pipeline(ntiles, emit_a, emit_b):
        # Interleave pass-A and trailing pass-B per tile so the PE's strict
        # FIFO always has B matmuls queued behind each A tile's score
        # matmuls — B work fills the STT-drain stall before the next A tile.
        groups = [range(g, min(g + G, ntiles)) for g in range(0, ntiles, G)]
        stash = {}
        for gi, grp in enumerate(groups):
            prev = list(groups[gi - 1]) if gi > 0 else []
            for i, t in enumerate(grp):
                stash[t] = emit_a(t)
                if i < len(prev):
                    emit_b(prev[i], stash.pop(prev[i]))
            for t in prev[len(grp):]:
                emit_b(t, stash.pop(t))
        for t in groups[-1]:
            emit_b(t, stash.pop(t))

    def pass_a2(t):
        """stage-2 scores: 4 concurrent row-tiled K=3 matmuls (one per
        512-support chunk); bit pattern verified identical on HW."""
        negd = ndp.tile([128, N1], F32, tag="negd")
        for gi in range(4):
            scch = ps_sc.tile([128, 512], F32, tag="sc")
            nc.tensor.matmul(out=scch[:],
                             lhsT=q2s[32 * gi:32 * gi + 3, ts(t, 128)],
                             rhs=r2s[32 * gi:32 * gi + 3, :],
                             tile_position=(32 * gi, 0),
                             start=True, stop=True)
            nc.vector.scalar_tensor_tensor(
                out=negd[:, 512 * gi:512 * (gi + 1)],
                in0=nb2[:, 512 * gi:512 * (gi + 1)],
                scalar=qn2s[:, t:t + 1], in1=scch[:],
                op0=ALU.add, op1=ALU.add,
            )
        return topk_tail(negd, t2, DT2)

    # ---- stage 1: 2048 queries x 512 supports -> t2[:, 256:768] ----
    pipeline(T1,
             lambda t: pass_a(t, q1s, r1s, nb1, qn1s, S1, x2t, D2),
             pass_b1)

    tc.strict_bb_all_engine_barrier()

    # ---- stage 2: 4096 queries x 2048 supports -> o2t ----
    pipeline(T2, pass_a2, pass_b2)


def build_program():
    from contextlib import ExitStack
    nc = bacc.Bacc("TRN2", target_bir_lowering=False, debug=False)
    with tile.TileContext(nc) as tc, ExitStack() as ctx:
        _emit(tc, ctx)
    nc.compile()
    return nc


def prep_core_inputs(xyz0, xyz1, xyz2, x0, x1, x2, core):
    b, h = divmod(core, 2)
    xyz1b = xyz1[b]
    xyz2b = xyz2[b]
    xyz0s = xyz0[b, h * NLOC:(h + 1) * NLOC]
    f32 = np.float32
    asc = np.ascontiguousarray

    def sumsq(v):                     # fp32 sequential, bitwise == jax sum(v*v)
        return ((v[:, 0] * v[:, 0] + v[:, 1] * v[:, 1]) + v[:, 2] * v[:, 2]).astype(f32)

    def qn(pts, ntiles):              # -|q|^2 arranged [128, ntiles]
        return asc((-sumsq(pts)).reshape(ntiles, 128).T)

    qrep2 = np.zeros((128, NLOC), f32)   # rows 32g+c = 2*coord c (all groups)
    rrep2 = np.zeros((128, 512), f32)    # rows 32g+c = coord c of chunk g
    for g in range(4):
        for c in range(3):
            qrep2[32 * g + c, :] = 2.0 * xyz0s[:, c]
            rrep2[32 * g + c, :] = xyz1b[512 * g:512 * (g + 1), c]

    return {
        "q1e": asc(2.0 * xyz1b.T).astype(f32), "r1": asc(xyz2b.T),
        "qn1": qn(xyz1b, T1),
        "b1r": asc(np.broadcast_to((-sumsq(xyz2b))[None, :], (128, S1))),
        "q2e": qrep2, "r2": rrep2,
        "qn2": qn(xyz0s, T2),
        "b2r": asc(np.broadcast_to((-sumsq(xyz1b))[None, :], (128, N1))),
        "x2t": asc(x2[b].T), "x1t": asc(x1[b].T),
    }


def run(inputs, trace=False):
    global _PROGRAM
    if _PROGRAM is None:
        _PROGRAM = build_program()
    in_maps = [prep_core_inputs(**inputs, core=c) for c in range(N_CORES)]
    return run_bass_kernel_spmd(
        _PROGRAM, in_maps, core_ids=list(range(N_CORES)), trace=trace,
    )


def assemble(inputs, results):
    out = np.empty((B, D0 + DT2, N0), np.float32)
    out[:, :D0, :] = inputs["x0"]
    for c in range(N_CORES):
        b, h = divmod(c, 2)
        out[b, D0:, h * NLOC:(h + 1) * NLOC] = results[c]["o2t"]
    return out


def kernel(**inputs):
    inputs = {k: np.asarray(v, np.float32) for k, v in inputs.items()}
    res = run(inputs)
    return assemble(inputs, res.results)
